# revision 15
# baseline (speedup 1.0000x reference)
"""Trainium2 Bass kernel for nn_CasualAttention_9474697855267.

Causal GQA attention block (B=2, S=2048, D=2048, H=16, H_KV=4, HD=128)
with RoPE, returning (out [B,S,D], attn [B,H,S,S]).

Sharding: tensor-parallel over heads across 8 NeuronCores. Core c owns
query heads {2c, 2c+1} (which share kv head c//2 under GQA). Each core:
  - projects q/k/v from the full x (shipped pre-transposed as xT),
  - applies RoPE,
  - computes causal softmax per head (scores scaled by 1/sqrt(HD) via the
    ScalarE exp's free affine), writes its attn slice,
  - computes o = attn @ v and a partial out = o @ Wo[rows of its heads].
Host sums the 8 partial outs and concatenates the attn slices.

Matmuls run as float32r (full fp32 storage; fast PE mode) accumulating in
fp32 PSUM. Softmax row-sums come for free from the ScalarE exp's
accum_out; diagonal blocks are masked with a 0/1 tril via a fused
tensor_tensor_reduce. Batches are processed sequentially so SBUF tags can
be reused (static tile-pool allocation).
"""

from contextlib import ExitStack

import numpy as np

B = 2
S = 2048
D = 2048
H = 16
H_KV = 4
HD = 128
NCORES = 8
HPC = H // NCORES  # 2 query heads per core
SCALE = HD ** -0.5
NEG = -1e9

T = B * S  # 4096 flattened tokens
QT = 128  # query tile rows
CG = 512  # key chunk columns (one PSUM bank of fp32)
NQT = S // QT  # 16 query tiles per (batch, head)
NKB = S // QT  # 16 key blocks
ND = D // 128  # 16 contraction chunks

# debug knobs (test-only; harness uses defaults)
_STAGES = "all"  # "proj" | "attn" | "all"
_USE_F32R = True
_ATTN_LEVEL = 5  # 1=scores/exp 2=+ttr 3=+norm/dma 4=+transpose/aT 5=+o-matmul


def _patched_tile_context_cls():
    """TileContext whose kernel-tail drain carries at most one sem wait.

    The stock _drain_and_barrier attaches a wait per logical proc (12 here)
    to a single SP Drain; this walrus rejects >2 sync waits on the CTRL
    instruction class ("Too many sync wait commands"). Absorb the global
    clock with a chain of single-wait SP NOPs instead.
    """
    import bass_rust
    import concourse.tile as tile
    from concourse.vector_clock import ScopedClock

    class PatchedTileContext(tile.TileContext):
        def _drain_and_barrier(self, tick_clock, wait_clock):
            nc = self.nc
            sink = nc.sync.nop()
            wait_clock.add_sem_waits(
                sink.ins, ScopedClock({None: tick_clock.global_clock})
            )
            si = sink.ins.sync_info
            waits = list(si.on_wait) if si is not None and si.on_wait else []
            if len(waits) > 1:
                sink.ins.sync_info = bass_rust.SyncInfo(
                    on_wait=[waits[0]], on_update=list(si.on_update or [])
                )
                for w in waits[1:]:
                    extra = nc.sync.nop()
                    extra.ins.sync_info = bass_rust.SyncInfo(on_wait=[w], on_update=[])
            nc.sync.drain()
            nc.all_engine_barrier()
            assert self.sems is not None
            popped = nc._tile_sem_poison_stack.pop()
            assert popped is self._sem_poison
            nc.clear_and_free_semaphores(list(self.sems.allocated().values()))
            nc.all_engine_barrier()

    return PatchedTileContext


def _build_nc(causal: bool):
    import concourse.bacc as bacc
    import concourse.mybir as mybir
    import concourse.tile as tile

    f32 = mybir.dt.float32
    f32r = mybir.dt.float32r if _USE_F32R else mybir.dt.float32
    AF = mybir.ActivationFunctionType
    ALU = mybir.AluOpType

    nc = bacc.Bacc("TRN2", target_bir_lowering=False, debug=False, num_devices=NCORES)

    # ---- I/O ----
    xT_d = nc.dram_tensor("xT", [D, T], f32r, kind="ExternalInput").ap()
    c_d = nc.dram_tensor("cosT", [HD, T], f32, kind="ExternalInput").ap()
    s_d = nc.dram_tensor("sinT", [HD, T], f32, kind="ExternalInput").ap()
    wq_d = nc.dram_tensor("wq", [D, HPC * HD], f32r, kind="ExternalInput").ap()
    wk_d = nc.dram_tensor("wk", [D, HD], f32r, kind="ExternalInput").ap()
    wv_d = nc.dram_tensor("wv", [D, HD], f32r, kind="ExternalInput").ap()
    wo_d = nc.dram_tensor("wo", [HPC * HD, D], f32r, kind="ExternalInput").ap()
    id_d = nc.dram_tensor("ident", [128, 128], f32, kind="ExternalInput").ap()
    tril_d = nc.dram_tensor("tril", [128, 128], f32, kind="ExternalInput").ap()
    if not causal:
        mask_d = nc.dram_tensor("mask", [B, S, S], f32, kind="ExternalInput").ap()

    attn_d = nc.dram_tensor("attn", [B, HPC, S, S], f32, kind="ExternalOutput").ap()
    out_d = nc.dram_tensor("out_partial", [T, D], f32, kind="ExternalOutput").ap()

    with tile.TileContext(nc) as tc, ExitStack() as ctx:
        pool = ctx.enter_context(tc.tile_pool(name="persist", bufs=1))
        xt_pool = ctx.enter_context(tc.tile_pool(name="xt", bufs=8))
        work = ctx.enter_context(tc.tile_pool(name="work", bufs=2))
        small = ctx.enter_context(tc.tile_pool(name="small", bufs=4))
        pp = ctx.enter_context(tc.tile_pool(name="pp", bufs=1, space="PSUM"))
        pp2 = ctx.enter_context(tc.tile_pool(name="pp2", bufs=2, space="PSUM"))

        # ---- constants (resident for the whole kernel) ----
        ident = pool.tile([128, 128], f32, tag="ident", name="ident")
        nc.sync.dma_start(out=ident[:], in_=id_d[:])
        tril = pool.tile([128, 128], f32, tag="tril", name="tril")
        nc.sync.dma_start(out=tril[:], in_=tril_d[:])
        zf = pool.tile([128, 128], f32, tag="zf", name="zf")
        nc.gpsimd.memset(zf[:], 0.0)
        wo_sb = []
        for j in range(HPC):
            t = pool.tile([128, D], f32r, tag=f"wo{j}", name=f"wo{j}")
            nc.sync.dma_start(out=t[:], in_=wo_d[j * 128 : (j + 1) * 128, :])
            wo_sb.append(t)

        def rope(dst, src_ps, cos_t, sin_t, n):
            """dst[:,:n] = rope(src_ps) with tables cos_t/sin_t [128, n]."""
            tmp = work.tile([128, CG], f32, tag="rope_tmp", name="rope_tmp")
            m1 = work.tile([128, CG], f32, tag="rope_m1", name="rope_m1")
            nc.vector.tensor_mul(tmp[:, :n], src_ps[:, :n], cos_t[:, :n])
            nc.vector.tensor_mul(m1[:64, :n], src_ps[64:128, :n], sin_t[:64, :n])
            nc.vector.tensor_mul(m1[64:128, :n], src_ps[:64, :n], sin_t[64:128, :n])
            nc.vector.tensor_sub(dst[:64, :n], tmp[:64, :n], m1[:64, :n])
            nc.vector.tensor_add(dst[64:128, :n], tmp[64:128, :n], m1[64:128, :n])

        for b in range(B):
            # ---- per-batch weights + tables (tags reused across batches) ----
            wq_sb, wk_sb, wv_sb = [], [], []
            for i in range(ND):
                # tag blk{i} is later reused by the attnT tiles
                t = pool.tile([128, CG], f32r, tag=f"blk{i}", name=f"wqc{i}")
                nc.sync.dma_start(
                    out=t[:, : HPC * HD], in_=wq_d[i * 128 : (i + 1) * 128, :]
                )
                wq_sb.append(t)
                t = pool.tile([128, HD], f32r, tag=f"wk{i}", name=f"wkc{i}")
                nc.sync.dma_start(out=t[:], in_=wk_d[i * 128 : (i + 1) * 128, :])
                wk_sb.append(t)
                t = pool.tile([128, HD], f32r, tag=f"wv{i}", name=f"wvc{i}")
                nc.sync.dma_start(out=t[:], in_=wv_d[i * 128 : (i + 1) * 128, :])
                wv_sb.append(t)
            cT = pool.tile([128, S], f32, tag="cT", name="cT")
            sT = pool.tile([128, S], f32, tag="sT", name="sT")
            nc.sync.dma_start(out=cT[:], in_=c_d[:, b * S : (b + 1) * S])
            nc.sync.dma_start(out=sT[:], in_=s_d[:, b * S : (b + 1) * S])

            qT = [pool.tile([128, S], f32r, tag=f"qT{j}", name=f"qT{j}") for j in range(HPC)]
            kT = pool.tile([128, S], f32r, tag="kT", name="kT")
            v_sb = [pool.tile([128, 128], f32r, tag=f"v{ki}", name=f"v{ki}") for ki in range(NKB)]
            oT = [pool.tile([128, S], f32r, tag=f"oT{j}", name=f"oT{j}") for j in range(HPC)]

            # ================= projection =================
            for g in range(S // CG):
                t0 = b * S + g * CG  # global token col
                tl = g * CG  # within-batch token col
                ps_q = [
                    pp.tile([128, CG], f32, tag=f"pq{j}", name=f"pq{j}")
                    for j in range(HPC)
                ]
                ps_k = pp.tile([128, CG], f32, tag="pk", name="pk")
                ps_v = pp.tile([128, CG], f32, tag="pv", name="pv")
                for i in range(ND):
                    xt = xt_pool.tile([128, CG], f32r, tag="xt", name="xt")
                    nc.sync.dma_start(
                        out=xt[:], in_=xT_d[i * 128 : (i + 1) * 128, t0 : t0 + CG]
                    )
                    st = i == 0
                    sp = i == ND - 1
                    for j in range(HPC):
                        nc.tensor.matmul(
                            ps_q[j][:],
                            wq_sb[i][:, j * 128 : (j + 1) * 128],
                            xt[:],
                            start=st,
                            stop=sp,
                        )
                    nc.tensor.matmul(ps_k[:], wk_sb[i][:], xt[:], start=st, stop=sp)
                    nc.tensor.matmul(ps_v[:], wv_sb[i][:], xt[:], start=st, stop=sp)

                for j in range(HPC):
                    rope(qT[j][:, tl : tl + CG], ps_q[j], cT[:, tl : tl + CG], sT[:, tl : tl + CG], CG)
                rope(kT[:, tl : tl + CG], ps_k, cT[:, tl : tl + CG], sT[:, tl : tl + CG], CG)

                # v: PSUM [hd, t] -> SBUF, then transpose to token-major blocks
                vtmp = work.tile([128, CG], f32, tag="vtmp", name="vtmp")
                nc.vector.tensor_copy(vtmp[:], ps_v[:])
                ptr = pp2.tile([128, CG], f32, tag="ptr", name="ptr_v")
                for u in range(CG // 128):
                    nc.tensor.transpose(
                        ptr[:, u * 128 : (u + 1) * 128],
                        vtmp[:, u * 128 : (u + 1) * 128],
                        ident[:],
                    )
                for u in range(CG // 128):
                    ki = (g * CG) // 128 + u
                    nc.scalar.copy(v_sb[ki][:], ptr[:, u * 128 : (u + 1) * 128])

            # ================= attention =================
            for j in range(HPC if _STAGES in ("attn", "all") else 0):
                cur_aT = {}
                for qi in range(NQT):
                    q0 = qi * QT
                    klen = q0 + QT if causal else S
                    ncg = (klen + CG - 1) // CG
                    g = qi // 4
                    u = qi % 4

                    e = work.tile([128, S], f32, tag="e", name="e")
                    sums = small.tile([128, 8], f32, tag="sums", name="sums")
                    ns = 0
                    for cg in range(ncg):
                        cc0 = cg * CG
                        clen = min(CG, klen - cc0)
                        ps = pp2.tile([128, CG], f32, tag="psc", name="ps_s")
                        nc.tensor.matmul(
                            ps[:, :clen],
                            qT[j][:, q0 : q0 + QT],
                            kT[:, cc0 : cc0 + clen],
                            start=True,
                            stop=True,
                        )
                        if not causal:
                            mt = work.tile([128, CG], f32, tag="mt", name="mt")
                            nc.sync.dma_start(
                                out=mt[:, :clen],
                                in_=mask_d[b, q0 : q0 + QT, cc0 : cc0 + clen],
                            )
                            ms = work.tile([128, CG], f32, tag="ms", name="ms")
                            nc.vector.tensor_scalar_mul(ms[:, :clen], ps[:, :clen], SCALE)
                            nc.vector.tensor_add(ms[:, :clen], ms[:, :clen], mt[:, :clen])
                            nc.scalar.activation(
                                e[:, cc0 : cc0 + clen],
                                ms[:, :clen],
                                AF.Exp,
                                accum_out=sums[:, ns : ns + 1],
                            )
                            ns += 1
                        elif cg < ncg - 1:
                            nc.scalar.activation(
                                e[:, cc0 : cc0 + clen],
                                ps[:, :clen],
                                AF.Exp,
                                scale=SCALE,
                                accum_out=sums[:, ns : ns + 1],
                            )
                            ns += 1
                        else:
                            # last chunk: [cc0, q0) prefix + diagonal 128 block
                            plen = q0 - cc0
                            if plen > 0:
                                nc.scalar.activation(
                                    e[:, cc0 : cc0 + plen],
                                    ps[:, :plen],
                                    AF.Exp,
                                    scale=SCALE,
                                    accum_out=sums[:, ns : ns + 1],
                                )
                                ns += 1
                            nc.scalar.activation(
                                e[:, q0 : q0 + QT],
                                ps[:, plen : plen + QT],
                                AF.Exp,
                                scale=SCALE,
                            )
                            if _ATTN_LEVEL < 2:
                                continue
                            nc.vector.tensor_mul(
                                e[:, q0 : q0 + QT], e[:, q0 : q0 + QT], tril[:]
                            )
                            nc.vector.reduce_sum(
                                sums[:, ns : ns + 1],
                                e[:, q0 : q0 + QT],
                                axis=mybir.AxisListType.X,
                            )
                            ns += 1

                    if _ATTN_LEVEL < 3:
                        continue
                    rs = small.tile([128, 1], f32, tag="rs", name="rs")
                    if ns > 1:
                        nc.vector.reduce_sum(rs[:], sums[:, :ns], axis=mybir.AxisListType.X)
                    else:
                        nc.vector.tensor_copy(rs[:], sums[:, :1])
                    rc = small.tile([128, 1], f32, tag="rc", name="rc")
                    nc.vector.reciprocal(rc[:], rs[:])

                    # normalize in place; e is now the attn row-block
                    nc.vector.tensor_scalar_mul(e[:, :klen], e[:, :klen], rc[:])
                    nc.sync.dma_start(
                        out=attn_d[b, j, q0 : q0 + QT, 0:klen], in_=e[:, :klen]
                    )

                    # transpose attn row-block into attnT tiles for o = attn @ v
                    if _ATTN_LEVEL < 4:
                        continue
                    nki = qi + 1 if causal else NKB
                    for k4 in range(0, nki, 4):
                        ptr = pp2.tile([128, CG], f32, tag="ptr", name="ptr_a")
                        kn = min(4, nki - k4)
                        for uu in range(kn):
                            ki = k4 + uu
                            nc.tensor.transpose(
                                ptr[:, uu * 128 : (uu + 1) * 128],
                                e[:, ki * 128 : (ki + 1) * 128],
                                ident[:],
                            )
                        for uu in range(kn):
                            ki = k4 + uu
                            if ki not in cur_aT:
                                at = pool.tile(
                                    [128, CG], f32r, tag=f"blk{ki}", name=f"aT{ki}"
                                )
                                cur_aT[ki] = at
                                # zero the qt column slices this tile will
                                # never receive (strictly-above-diagonal)
                                for u2 in range(max(0, min(ki - 4 * g, 4))):
                                    nc.vector.tensor_copy(
                                        at[:, u2 * 128 : (u2 + 1) * 128], zf[:]
                                    )
                            nc.scalar.copy(
                                cur_aT[ki][:, u * 128 : (u + 1) * 128],
                                ptr[:, uu * 128 : (uu + 1) * 128],
                            )

                    if u == 3 and _ATTN_LEVEL >= 5:
                        nki_g = 4 * g + 4 if causal else NKB
                        po = pp.tile([128, CG], f32, tag="pq0", name="po")
                        for ki in range(nki_g):
                            nc.tensor.matmul(
                                po[:],
                                v_sb[ki][:],
                                cur_aT[ki][:],
                                start=(ki == 0),
                                stop=(ki == nki_g - 1),
                            )
                        nc.vector.tensor_copy(oT[j][:, g * CG : (g + 1) * CG], po[:])
                        cur_aT = {}

            # ================= output projection (partial) =================
            for ti in range(S // 128 if _STAGES == "all" else 0):
                ot = work.tile([128, D], f32, tag="ot", name="ot")
                for cg in range(D // CG):
                    pw = pp.tile([128, CG], f32, tag="pq1", name="pw")
                    for j in range(HPC):
                        nc.tensor.matmul(
                            pw[:],
                            oT[j][:, ti * 128 : (ti + 1) * 128],
                            wo_sb[j][:, cg * CG : (cg + 1) * CG],
                            start=(j == 0),
                            stop=(j == HPC - 1),
                        )
                    if cg % 2 == 0:
                        nc.vector.tensor_copy(ot[:, cg * CG : (cg + 1) * CG], pw[:])
                    else:
                        nc.scalar.copy(ot[:, cg * CG : (cg + 1) * CG], pw[:])
                r0 = b * S + ti * 128
                nc.sync.dma_start(out=out_d[r0 : r0 + 128, :], in_=ot[:])

    nc.compile()
    return nc


_NC_CACHE = {}


def _get_nc(causal: bool):
    key = (causal, _STAGES, _USE_F32R, _ATTN_LEVEL)
    if key not in _NC_CACHE:
        _NC_CACHE[key] = _build_nc(causal)
    return _NC_CACHE[key]


def _is_causal_mask(mask: np.ndarray) -> bool:
    if mask.shape != (B, 1, S, S):
        return False
    expect = np.where(
        np.tril(np.ones((S, S), dtype=bool)), np.float32(0.0), np.float32(NEG)
    )
    return all(np.array_equal(mask[b, 0], expect) for b in range(B))


def _prep_in_maps(x, cos, sin, attention_mask, Wq, Wk, Wv, Wo, causal):
    xT = np.ascontiguousarray(x.reshape(T, D).T)
    cosT = np.ascontiguousarray(cos.reshape(T, HD).T)
    sinT = np.ascontiguousarray(sin.reshape(T, HD).T)
    ident = np.eye(128, dtype=np.float32)
    tril = np.tril(np.ones((128, 128), dtype=np.float32))

    in_maps = []
    for c in range(NCORES):
        kvh = c // 2
        im = {
            "xT": xT,
            "cosT": cosT,
            "sinT": sinT,
            "wq": np.ascontiguousarray(Wq[:, c * HPC * HD : (c + 1) * HPC * HD]),
            "wk": np.ascontiguousarray(Wk[:, kvh * HD : (kvh + 1) * HD]),
            "wv": np.ascontiguousarray(Wv[:, kvh * HD : (kvh + 1) * HD]),
            "wo": np.ascontiguousarray(Wo[c * HPC * HD : (c + 1) * HPC * HD, :]),
            "ident": ident,
            "tril": tril,
        }
        if not causal:
            im["mask"] = np.ascontiguousarray(
                np.broadcast_to(attention_mask[:, 0], (B, S, S))
            )
        in_maps.append(im)
    return in_maps


def kernel(x, cos, sin, attention_mask, Wq, Wk, Wv, Wo):
    from concourse.bass_utils import run_bass_kernel_spmd

    x = np.asarray(x, dtype=np.float32)
    cos = np.asarray(cos, dtype=np.float32)
    sin = np.asarray(sin, dtype=np.float32)
    attention_mask = np.asarray(attention_mask, dtype=np.float32)
    Wq = np.asarray(Wq, dtype=np.float32)
    Wk = np.asarray(Wk, dtype=np.float32)
    Wv = np.asarray(Wv, dtype=np.float32)
    Wo = np.asarray(Wo, dtype=np.float32)

    causal = _is_causal_mask(attention_mask)
    nc = _get_nc(causal)
    in_maps = _prep_in_maps(x, cos, sin, attention_mask, Wq, Wk, Wv, Wo, causal)

    res = run_bass_kernel_spmd(nc, in_maps, list(range(NCORES))).results

    attn = np.concatenate([res[c]["attn"] for c in range(NCORES)], axis=1)
    out = res[0]["out_partial"].astype(np.float32)
    for c in range(1, NCORES):
        out = out + res[c]["out_partial"]
    out = out.reshape(B, S, D)
    return out, attn


# revision 18
# speedup vs baseline: 1.0859x; 1.0859x over previous
"""Trainium2 Bass kernel for nn_CasualAttention_9474697855267.

Causal GQA attention block (B=2, S=2048, D=2048, H=16, H_KV=4, HD=128)
with RoPE, returning (out [B,S,D], attn [B,H,S,S]).

Sharding: tensor-parallel over heads across 8 NeuronCores. Core c owns
query heads {2c, 2c+1} (which share kv head c//2 under GQA). Each core
projects q/k/v from the full x (shipped pre-transposed as xT), applies
RoPE, computes causal softmax per head, and produces its attn slice plus
a partial out = (attn @ v) @ Wo[rows of its heads]. The host sums the 8
partial outs and concatenates/transposes the attn slices.

The attention matrix is computed TRANSPOSED on device (scoresT[kt,qt] =
k @ qT directly from the projection layouts), so no PE transposes or
PSUM->SBUF copies are needed between softmax and the o = attn @ v
matmul: the ScalarE exp writes the o-matmul operand in place. Row sums
are accumulated with a ones-vector matmul on the TensorE; the diagonal
blocks are masked by a fused multiply with host-provided composite mask
tiles. The device writes attn in [kt, qt] layout; the host transposes.

Matmuls run as float32r (fp32 storage, reduced-precision fast PE mode)
accumulating in fp32 PSUM.
"""

from contextlib import ExitStack

import numpy as np

B = 2
S = 2048
D = 2048
H = 16
H_KV = 4
HD = 128
NCORES = 8
HPC = H // NCORES  # 2 query heads per core
SCALE = HD ** -0.5
NEG = -1e9

T = B * S  # 4096 flattened tokens
QG = 512  # query columns per attention group (PSUM bank of fp32)
NG = S // QG  # 4 query groups per (batch, head)
NKB = S // 128  # 16 key blocks
ND = D // 128  # 16 contraction chunks

# debug knobs (test-only; harness uses defaults)
_STAGES = "all"  # "proj" | "attn" | "all"


def _build_nc(causal: bool):
    import concourse.bacc as bacc
    import concourse.mybir as mybir
    import concourse.tile as tile

    f32 = mybir.dt.float32
    f32r = mybir.dt.float32r
    AF = mybir.ActivationFunctionType

    nc = bacc.Bacc("TRN2", target_bir_lowering=False, debug=False, num_devices=NCORES)

    # ---- I/O ----
    xT_d = nc.dram_tensor("xT", [D, T], f32r, kind="ExternalInput").ap()
    c_d = nc.dram_tensor("cosT", [HD, T], f32, kind="ExternalInput").ap()
    s_d = nc.dram_tensor("sinT", [HD, T], f32, kind="ExternalInput").ap()
    wq_d = nc.dram_tensor("wq", [D, HPC * HD], f32r, kind="ExternalInput").ap()
    wk_d = nc.dram_tensor("wk", [D, HD], f32r, kind="ExternalInput").ap()
    wv_d = nc.dram_tensor("wv", [D, HD], f32r, kind="ExternalInput").ap()
    wo_d = nc.dram_tensor("wo", [HPC * HD, D], f32r, kind="ExternalInput").ap()
    id_d = nc.dram_tensor("ident", [128, 128], f32, kind="ExternalInput").ap()
    ones_d = nc.dram_tensor("ones", [128, 1], f32r, kind="ExternalInput").ap()
    # composite diagonal masks: dmask[p] for a kt-block p sub-positions into
    # its qt group: sub-block u: u>p -> ones, u==p -> triu(incl diag), u<p -> 0
    dm_d = nc.dram_tensor("dmask", [4, 128, QG], f32, kind="ExternalInput").ap()
    if not causal:
        # maskT[b, kt, qt] = attention_mask[b, qt, kt] / SCALE
        mask_d = nc.dram_tensor("maskT", [B, S, S], f32, kind="ExternalInput").ap()

    # attn in TRANSPOSED per-head layout [kt, qt]; host swaps the last axes
    attn_d = nc.dram_tensor("attnT", [B, HPC, S, S], f32r, kind="ExternalOutput").ap()
    out_d = nc.dram_tensor("out_partial", [T, D], f32, kind="ExternalOutput").ap()

    with tile.TileContext(nc) as tc, ExitStack() as ctx:
        pool = ctx.enter_context(tc.tile_pool(name="persist", bufs=1))
        xt_pool = ctx.enter_context(tc.tile_pool(name="xt", bufs=4))
        work = ctx.enter_context(tc.tile_pool(name="work", bufs=2))
        small = ctx.enter_context(tc.tile_pool(name="small", bufs=4))
        pp = ctx.enter_context(tc.tile_pool(name="pp", bufs=2, space="PSUM"))

        # ---- constants + weights (resident for the whole kernel) ----
        ident = pool.tile([128, 128], f32, tag="ident", name="ident")
        nc.sync.dma_start(out=ident[:], in_=id_d[:])
        ones = pool.tile([128, 1], f32r, tag="ones", name="ones")
        nc.sync.dma_start(out=ones[:], in_=ones_d[:])
        ones_row = pool.tile([1, 128], f32r, tag="ones_row", name="ones_row")
        nc.sync.dma_start(out=ones_row[:], in_=ones_d[:].rearrange("p o -> o p"))
        dmask = pool.tile([128, 4 * QG], f32, tag="dmask", name="dmask")
        for p in range(4):
            nc.sync.dma_start(out=dmask[:, p * QG : (p + 1) * QG], in_=dm_d[p])

        wq_sb, wk_sb, wv_sb = [], [], []
        for i in range(ND):
            t = pool.tile([128, HPC * HD], f32r, tag=f"wq{i}", name=f"wq{i}")
            nc.sync.dma_start(out=t[:], in_=wq_d[i * 128 : (i + 1) * 128, :])
            wq_sb.append(t)
            t = pool.tile([128, HD], f32r, tag=f"wk{i}", name=f"wk{i}")
            nc.sync.dma_start(out=t[:], in_=wk_d[i * 128 : (i + 1) * 128, :])
            wk_sb.append(t)
            t = pool.tile([128, HD], f32r, tag=f"wv{i}", name=f"wv{i}")
            nc.sync.dma_start(out=t[:], in_=wv_d[i * 128 : (i + 1) * 128, :])
            wv_sb.append(t)
        wo_sb = []
        for j in range(HPC):
            t = pool.tile([128, D], f32r, tag=f"wo{j}", name=f"wo{j}")
            nc.sync.dma_start(out=t[:], in_=wo_d[j * 128 : (j + 1) * 128, :])
            wo_sb.append(t)

        def rope(dst, src_ps, cos_t, sin_t):
            """dst[:, :512] = rope(src_ps[:, :512]) with [128, 512] tables."""
            n = QG
            tmp = work.tile([128, QG], f32, tag="rope_tmp", name="rope_tmp")
            m1 = work.tile([128, QG], f32, tag="rope_m1", name="rope_m1")
            nc.vector.tensor_mul(tmp[:, :n], src_ps[:, :n], cos_t[:, :n])
            nc.vector.tensor_mul(m1[:64, :n], src_ps[64:128, :n], sin_t[:64, :n])
            nc.vector.tensor_mul(m1[64:128, :n], src_ps[:64, :n], sin_t[64:128, :n])
            nc.vector.tensor_sub(dst[:64, :n], tmp[:64, :n], m1[:64, :n])
            nc.vector.tensor_add(dst[64:128, :n], tmp[64:128, :n], m1[64:128, :n])

        for b in range(B):
            cT = pool.tile([128, S], f32, tag="cT", name="cT")
            sT = pool.tile([128, S], f32, tag="sT", name="sT")
            nc.sync.dma_start(out=cT[:], in_=c_d[:, b * S : (b + 1) * S])
            nc.sync.dma_start(out=sT[:], in_=s_d[:, b * S : (b + 1) * S])

            qT = [pool.tile([128, S], f32r, tag=f"qT{j}", name=f"qT{j}") for j in range(HPC)]
            kT = pool.tile([128, S], f32r, tag="kT", name="kT")
            v_sb = [pool.tile([128, 128], f32r, tag=f"v{ki}", name=f"v{ki}") for ki in range(NKB)]
            oT = [pool.tile([128, S], f32r, tag=f"oT{j}", name=f"oT{j}") for j in range(HPC)]

            # ================= projection =================
            # two passes per 1024-token slab: (q0,q1) then (k,v); each weight
            # chunk is loaded once and used for two 512-wide matmuls.
            for g2 in range(S // 1024):
                t0 = b * S + g2 * 1024
                tl = g2 * 1024
                xts = []
                for i in range(ND):
                    xt = xt_pool.tile([128, 1024], f32r, tag="xt", name="xt")
                    nc.sync.dma_start(
                        out=xt[:], in_=xT_d[i * 128 : (i + 1) * 128, t0 : t0 + 1024]
                    )
                    xts.append(xt)
                # q pass
                ps = [
                    pp.tile([128, QG], f32, tag=f"P{k}", name=f"psq{k}")
                    for k in range(4)
                ]
                for i in range(ND):
                    st, sp = i == 0, i == ND - 1
                    for j in range(HPC):
                        for h in range(2):
                            nc.tensor.matmul(
                                ps[2 * j + h][:],
                                wq_sb[i][:, j * 128 : (j + 1) * 128],
                                xts[i][:, h * QG : (h + 1) * QG],
                                start=st,
                                stop=sp,
                            )
                for j in range(HPC):
                    for h in range(2):
                        c0 = tl + h * QG
                        rope(qT[j][:, c0 : c0 + QG], ps[2 * j + h], cT[:, c0 : c0 + QG], sT[:, c0 : c0 + QG])
                # kv pass
                ps = [
                    pp.tile([128, QG], f32, tag=f"P{k}", name=f"pskv{k}")
                    for k in range(4)
                ]
                for i in range(ND):
                    st, sp = i == 0, i == ND - 1
                    for h in range(2):
                        nc.tensor.matmul(
                            ps[h][:],
                            wk_sb[i][:],
                            xts[i][:, h * QG : (h + 1) * QG],
                            start=st,
                            stop=sp,
                        )
                        nc.tensor.matmul(
                            ps[2 + h][:],
                            wv_sb[i][:],
                            xts[i][:, h * QG : (h + 1) * QG],
                            start=st,
                            stop=sp,
                        )
                for h in range(2):
                    c0 = tl + h * QG
                    rope(kT[:, c0 : c0 + QG], ps[h], cT[:, c0 : c0 + QG], sT[:, c0 : c0 + QG])
                # v: PSUM [hd, t] -> SBUF -> token-major blocks via PE transpose
                for h in range(2):
                    vtmp = work.tile([128, QG], f32, tag="vtmp", name="vtmp")
                    nc.vector.tensor_copy(vtmp[:], ps[2 + h][:])
                    ptr = pp.tile([128, QG], f32, tag="P3", name="ptr_v")
                    for u in range(QG // 128):
                        nc.tensor.transpose(
                            ptr[:, u * 128 : (u + 1) * 128],
                            vtmp[:, u * 128 : (u + 1) * 128],
                            ident[:],
                        )
                    for u in range(QG // 128):
                        ki = (g2 * 1024 + h * QG) // 128 + u
                        nc.scalar.copy(v_sb[ki][:], ptr[:, u * 128 : (u + 1) * 128])

            # ================= attention (transposed) =================
            if _STAGES in ("attn", "all"):
                for j in range(HPC):
                    for g in range(NG):
                        q0 = g * QG
                        nki = 4 * g + 4 if causal else NKB
                        aT = pool.tile([128, NKB * QG], f32r, tag="aT", name="aT")
                        rs = pp.tile([1, QG], f32, tag="P1", name="rs")
                        for ki in range(nki):
                            sc = pp.tile(
                                [128, QG], f32, tag=f"P{2 + ki % 2}", name="sc"
                            )
                            nc.tensor.matmul(
                                sc[:],
                                kT[:, ki * 128 : (ki + 1) * 128],
                                qT[j][:, q0 : q0 + QG],
                                start=True,
                                stop=True,
                            )
                            dst = aT[:, ki * QG : (ki + 1) * QG]
                            if causal:
                                nc.scalar.activation(dst, sc[:], AF.Exp, scale=SCALE)
                                if ki >= 4 * g:
                                    p = ki - 4 * g
                                    nc.vector.tensor_mul(
                                        dst, dst, dmask[:, p * QG : (p + 1) * QG]
                                    )
                            else:
                                mt = work.tile([128, QG], f32, tag="mt", name="mt")
                                nc.sync.dma_start(
                                    out=mt[:],
                                    in_=mask_d[b, ki * 128 : (ki + 1) * 128, q0 : q0 + QG],
                                )
                                ms = work.tile([128, QG], f32, tag="ms", name="ms")
                                nc.vector.tensor_add(ms[:], sc[:], mt[:])
                                nc.scalar.activation(dst, ms[:], AF.Exp, scale=SCALE)
                        # row sums via ones-vector matmul, then reciprocal
                        for ki in range(nki):
                            nc.tensor.matmul(
                                rs[:],
                                ones[:],
                                aT[:, ki * QG : (ki + 1) * QG],
                                start=(ki == 0),
                                stop=(ki == nki - 1),
                            )
                        rc = small.tile([1, QG], f32r, tag="rc", name="rc")
                        with nc.allow_low_precision(reason="f32r softmax recip"):
                            nc.vector.reciprocal(rc[:], rs[:])
                        # broadcast recip across partitions via K=1 outer product
                        rb = pp.tile([128, QG], f32, tag="P1", name="rb")
                        nc.tensor.matmul(rb[:], ones_row[:], rc[:], start=True, stop=True)
                        for ki in range(nki):
                            dst = aT[:, ki * QG : (ki + 1) * QG]
                            nc.vector.tensor_mul(dst, dst, rb[:])
                        # one strided DMA writes the whole group's attn slice
                        nc.sync.dma_start(
                            out=attn_d[b, j, 0 : nki * 128, q0 : q0 + QG].rearrange(
                                "(k p) c -> p k c", p=128
                            ),
                            in_=aT[:, : nki * QG].rearrange("p (k c) -> p k c", c=QG),
                        )
                        # o^T accumulation: oT[j][:, q0:q0+QG] += v[ki].T @ aT[ki]
                        po = pp.tile([128, QG], f32, tag="P0", name="po")
                        for ki in range(nki):
                            nc.tensor.matmul(
                                po[:],
                                v_sb[ki][:],
                                aT[:, ki * QG : (ki + 1) * QG],
                                start=(ki == 0),
                                stop=(ki == nki - 1),
                            )
                        nc.vector.tensor_copy(oT[j][:, q0 : q0 + QG], po[:])

            # ================= output projection (partial) =================
            if _STAGES == "all":
                for ti in range(S // 128):
                    ot = work.tile([128, D], f32, tag="ot", name="ot")
                    pw = [
                        pp.tile([128, QG], f32, tag=f"P{k}", name=f"pw{k}")
                        for k in range(4)
                    ]
                    for j in range(HPC):
                        for cg in range(D // QG):
                            nc.tensor.matmul(
                                pw[cg][:],
                                oT[j][:, ti * 128 : (ti + 1) * 128],
                                wo_sb[j][:, cg * QG : (cg + 1) * QG],
                                start=(j == 0),
                                stop=(j == HPC - 1),
                            )
                    for cg in range(D // QG):
                        if cg % 2 == 0:
                            nc.vector.tensor_copy(ot[:, cg * QG : (cg + 1) * QG], pw[cg][:])
                        else:
                            nc.scalar.copy(ot[:, cg * QG : (cg + 1) * QG], pw[cg][:])
                    r0 = b * S + ti * 128
                    nc.sync.dma_start(out=out_d[r0 : r0 + 128, :], in_=ot[:])

    nc.compile()
    return nc


_NC_CACHE = {}


def _get_nc(causal: bool):
    key = (causal, _STAGES)
    if key not in _NC_CACHE:
        _NC_CACHE[key] = _build_nc(causal)
    return _NC_CACHE[key]


def _is_causal_mask(mask: np.ndarray) -> bool:
    if mask.shape != (B, 1, S, S):
        return False
    expect = np.where(
        np.tril(np.ones((S, S), dtype=bool)), np.float32(0.0), np.float32(NEG)
    )
    return all(np.array_equal(mask[b, 0], expect) for b in range(B))


def _make_dmask():
    """dmask[p][r, u*128 + c] for kt sub-position p vs qt sub-block u."""
    dm = np.zeros((4, 128, QG), dtype=np.float32)
    triu = np.triu(np.ones((128, 128), dtype=np.float32))  # keep kt<=qt
    for p in range(4):
        for u in range(4):
            if p < u:
                dm[p][:, u * 128 : (u + 1) * 128] = 1.0
            elif p == u:
                dm[p][:, u * 128 : (u + 1) * 128] = triu
    return dm


def _prep_in_maps(x, cos, sin, attention_mask, Wq, Wk, Wv, Wo, causal):
    xT = np.ascontiguousarray(x.reshape(T, D).T)
    cosT = np.ascontiguousarray(cos.reshape(T, HD).T)
    sinT = np.ascontiguousarray(sin.reshape(T, HD).T)
    ident = np.eye(128, dtype=np.float32)
    ones = np.ones((128, 1), dtype=np.float32)
    dmask = _make_dmask()

    in_maps = []
    for c in range(NCORES):
        kvh = c // 2
        im = {
            "xT": xT,
            "cosT": cosT,
            "sinT": sinT,
            "wq": np.ascontiguousarray(Wq[:, c * HPC * HD : (c + 1) * HPC * HD]),
            "wk": np.ascontiguousarray(Wk[:, kvh * HD : (kvh + 1) * HD]),
            "wv": np.ascontiguousarray(Wv[:, kvh * HD : (kvh + 1) * HD]),
            "wo": np.ascontiguousarray(Wo[c * HPC * HD : (c + 1) * HPC * HD, :]),
            "ident": ident,
            "ones": ones,
            "dmask": dmask,
        }
        if not causal:
            im["maskT"] = np.ascontiguousarray(
                np.swapaxes(attention_mask[:, 0], 1, 2) / np.float32(SCALE)
            )
        in_maps.append(im)
    return in_maps


def kernel(x, cos, sin, attention_mask, Wq, Wk, Wv, Wo):
    from concourse.bass_utils import run_bass_kernel_spmd

    x = np.asarray(x, dtype=np.float32)
    cos = np.asarray(cos, dtype=np.float32)
    sin = np.asarray(sin, dtype=np.float32)
    attention_mask = np.asarray(attention_mask, dtype=np.float32)
    Wq = np.asarray(Wq, dtype=np.float32)
    Wk = np.asarray(Wk, dtype=np.float32)
    Wv = np.asarray(Wv, dtype=np.float32)
    Wo = np.asarray(Wo, dtype=np.float32)

    causal = _is_causal_mask(attention_mask)
    nc = _get_nc(causal)
    in_maps = _prep_in_maps(x, cos, sin, attention_mask, Wq, Wk, Wv, Wo, causal)

    res = run_bass_kernel_spmd(nc, in_maps, list(range(NCORES))).results

    attn = np.concatenate(
        [np.swapaxes(res[c]["attnT"], 2, 3) for c in range(NCORES)], axis=1
    )
    attn = np.ascontiguousarray(attn)
    out = res[0]["out_partial"].astype(np.float32)
    for c in range(1, NCORES):
        out = out + res[c]["out_partial"]
    out = out.reshape(B, S, D)
    return out, attn


# revision 22
# speedup vs baseline: 1.1046x; 1.0173x over previous
"""Trainium2 Bass kernel for nn_CasualAttention_9474697855267.

Causal GQA attention block (B=2, S=2048, D=2048, H=16, H_KV=4, HD=128)
with RoPE, returning (out [B,S,D], attn [B,H,S,S]).

Sharding: tensor-parallel over heads across 8 NeuronCores. Core c owns
query heads {2c, 2c+1} (which share kv head c//2 under GQA). Each core
projects q/k/v from the full x (shipped pre-transposed as xT), applies
RoPE, computes causal softmax per head, and produces its attn slice plus
a partial out = (attn @ v) @ Wo[rows of its heads]. The host sums the 8
partial outs and concatenates/transposes the attn slices.

The attention matrix is computed TRANSPOSED on device (scoresT[kt,qt] =
k @ qT directly from the projection layouts), so no PE transposes or
PSUM->SBUF copies are needed between softmax and the o = attn @ v
matmul: the ScalarE exp writes the o-matmul operand in place. Row sums
are accumulated with a ones-vector matmul on the TensorE; the diagonal
blocks are masked by a fused multiply with host-provided composite mask
tiles. The device writes attn in [kt, qt] layout; the host transposes.

Matmuls run as float32r (fp32 storage, reduced-precision fast PE mode)
accumulating in fp32 PSUM.
"""

from contextlib import ExitStack

import numpy as np

B = 2
S = 2048
D = 2048
H = 16
H_KV = 4
HD = 128
NCORES = 8
HPC = H // NCORES  # 2 query heads per core
SCALE = HD ** -0.5
NEG = -1e9

T = B * S  # 4096 flattened tokens
QG = 512  # query columns per attention group (PSUM bank of fp32)
NG = S // QG  # 4 query groups per (batch, head)
NKB = S // 128  # 16 key blocks
ND = D // 128  # 16 contraction chunks

# debug knobs (test-only; harness uses defaults)
_STAGES = "all"  # "proj" | "attn" | "all"


def _build_nc(causal: bool):
    import concourse.bacc as bacc
    import concourse.mybir as mybir
    import concourse.tile as tile

    f32 = mybir.dt.float32
    f32r = mybir.dt.float32r
    AF = mybir.ActivationFunctionType

    nc = bacc.Bacc("TRN2", target_bir_lowering=False, debug=False, num_devices=NCORES, dynamic_dma_scratch_size=512)

    # ---- I/O ----
    xT_d = nc.dram_tensor("xT", [D, T], f32r, kind="ExternalInput").ap()
    c_d = nc.dram_tensor("cosT", [HD, T], f32, kind="ExternalInput").ap()
    s_d = nc.dram_tensor("sinT", [HD, T], f32, kind="ExternalInput").ap()
    wq_d = nc.dram_tensor("wq", [D, HPC * HD], f32r, kind="ExternalInput").ap()
    wk_d = nc.dram_tensor("wk", [D, HD], f32r, kind="ExternalInput").ap()
    wv_d = nc.dram_tensor("wv", [D, HD], f32r, kind="ExternalInput").ap()
    wo_d = nc.dram_tensor("wo", [HPC * HD, D], f32r, kind="ExternalInput").ap()
    id_d = nc.dram_tensor("ident", [128, 128], f32, kind="ExternalInput").ap()
    ones_d = nc.dram_tensor("ones", [128, 128], f32r, kind="ExternalInput").ap()
    # composite diagonal masks: dmask[p] for a kt-block p sub-positions into
    # its qt group: sub-block u: u>p -> ones, u==p -> triu(incl diag), u<p -> 0
    dm_d = nc.dram_tensor("dmask", [4, 128, QG], f32, kind="ExternalInput").ap()
    if not causal:
        # maskT[b, kt, qt] = attention_mask[b, qt, kt] / SCALE
        mask_d = nc.dram_tensor("maskT", [B, S, S], f32, kind="ExternalInput").ap()

    # attn in TRANSPOSED per-head layout [kt, qt]; host swaps the last axes
    attn_d = nc.dram_tensor("attnT", [B, HPC, S, S], f32r, kind="ExternalOutput").ap()
    out_d = nc.dram_tensor("out_partial", [T, D], f32, kind="ExternalOutput").ap()

    with tile.TileContext(nc) as tc, ExitStack() as ctx:
        pool = ctx.enter_context(tc.tile_pool(name="persist", bufs=1))
        xt_pool = ctx.enter_context(tc.tile_pool(name="xt", bufs=4))
        work = ctx.enter_context(tc.tile_pool(name="work", bufs=2))
        small = ctx.enter_context(tc.tile_pool(name="small", bufs=4))
        pp = ctx.enter_context(tc.tile_pool(name="pp", bufs=2, space="PSUM"))

        # ---- constants + weights (resident for the whole kernel) ----
        ident = pool.tile([128, 128], f32, tag="ident", name="ident")
        nc.sync.dma_start(out=ident[:], in_=id_d[:])
        ones = pool.tile([128, 128], f32r, tag="ones", name="ones")
        nc.sync.dma_start(out=ones[:], in_=ones_d[:])
        dmask = pool.tile([128, 4 * QG], f32, tag="dmask", name="dmask")
        for p in range(4):
            nc.sync.dma_start(out=dmask[:, p * QG : (p + 1) * QG], in_=dm_d[p])

        wq_sb, wk_sb, wv_sb = [], [], []
        for i in range(ND):
            t = pool.tile([128, HPC * HD], f32r, tag=f"wq{i}", name=f"wq{i}")
            nc.sync.dma_start(out=t[:], in_=wq_d[i * 128 : (i + 1) * 128, :])
            wq_sb.append(t)
            t = pool.tile([128, HD], f32r, tag=f"wk{i}", name=f"wk{i}")
            nc.sync.dma_start(out=t[:], in_=wk_d[i * 128 : (i + 1) * 128, :])
            wk_sb.append(t)
            t = pool.tile([128, HD], f32r, tag=f"wv{i}", name=f"wv{i}")
            nc.sync.dma_start(out=t[:], in_=wv_d[i * 128 : (i + 1) * 128, :])
            wv_sb.append(t)
        wo_sb = []
        for j in range(HPC):
            t = pool.tile([128, D], f32r, tag=f"wo{j}", name=f"wo{j}")
            nc.sync.dma_start(out=t[:], in_=wo_d[j * 128 : (j + 1) * 128, :])
            wo_sb.append(t)

        def rope(dst, src_ps, cos_t, sin_t):
            """dst[:, :512] = rope(src_ps[:, :512]) with [128, 512] tables."""
            n = QG
            tmp = work.tile([128, QG], f32, tag="rope_tmp", name="rope_tmp", bufs=1)
            m1 = work.tile([128, QG], f32, tag="rope_m1", name="rope_m1", bufs=1)
            nc.vector.tensor_mul(tmp[:, :n], src_ps[:, :n], cos_t[:, :n])
            nc.vector.tensor_mul(m1[:64, :n], src_ps[64:128, :n], sin_t[:64, :n])
            nc.vector.tensor_mul(m1[64:128, :n], src_ps[:64, :n], sin_t[64:128, :n])
            nc.vector.tensor_sub(dst[:64, :n], tmp[:64, :n], m1[:64, :n])
            nc.vector.tensor_add(dst[64:128, :n], tmp[64:128, :n], m1[64:128, :n])

        for b in range(B):
            cT = pool.tile([128, S], f32, tag="cT", name="cT")
            sT = pool.tile([128, S], f32, tag="sT", name="sT")
            nc.sync.dma_start(out=cT[:], in_=c_d[:, b * S : (b + 1) * S])
            nc.sync.dma_start(out=sT[:], in_=s_d[:, b * S : (b + 1) * S])

            qT = [pool.tile([128, S], f32r, tag=f"qT{j}", name=f"qT{j}") for j in range(HPC)]
            kT = pool.tile([128, S], f32r, tag="kT", name="kT")
            v_sb = [pool.tile([128, 128], f32r, tag=f"v{ki}", name=f"v{ki}") for ki in range(NKB)]
            oT = [pool.tile([128, S], f32r, tag=f"oT{j}", name=f"oT{j}") for j in range(HPC)]

            # ================= projection =================
            # single pass per 1024-token slab: 8 concurrent PSUM accumulators
            # (q0,q1,k,v) x (2 halves) so xt tiles stream once
            for g2 in range(S // 1024):
                t0 = b * S + g2 * 1024
                tl = g2 * 1024
                ps = [
                    pp.tile([128, QG], f32, tag=f"P{k % 4}", name=f"psp{k}")
                    for k in range(8)
                ]
                for i in range(ND):
                    xt = xt_pool.tile([128, 1024], f32r, tag="xt", name="xt")
                    nc.sync.dma_start(
                        out=xt[:], in_=xT_d[i * 128 : (i + 1) * 128, t0 : t0 + 1024]
                    )
                    st, sp = i == 0, i == ND - 1
                    for h in range(2):
                        xh = xt[:, h * QG : (h + 1) * QG]
                        for j in range(HPC):
                            nc.tensor.matmul(
                                ps[4 * h + j][:],
                                wq_sb[i][:, j * 128 : (j + 1) * 128],
                                xh,
                                start=st,
                                stop=sp,
                            )
                        nc.tensor.matmul(ps[4 * h + 2][:], wk_sb[i][:], xh, start=st, stop=sp)
                        nc.tensor.matmul(ps[4 * h + 3][:], wv_sb[i][:], xh, start=st, stop=sp)
                for h in range(2):
                    c0 = tl + h * QG
                    for j in range(HPC):
                        rope(qT[j][:, c0 : c0 + QG], ps[4 * h + j], cT[:, c0 : c0 + QG], sT[:, c0 : c0 + QG])
                    rope(kT[:, c0 : c0 + QG], ps[4 * h + 2], cT[:, c0 : c0 + QG], sT[:, c0 : c0 + QG])
                    # v: PSUM [hd, t] -> SBUF -> token-major blocks via PE transpose
                    vtmp = work.tile([128, QG], f32, tag="vtmp", name="vtmp", bufs=1)
                    nc.vector.tensor_copy(vtmp[:], ps[4 * h + 3][:])
                    ptr = pp.tile([128, QG], f32, tag="P3", name="ptr_v")
                    for u in range(QG // 128):
                        nc.tensor.transpose(
                            ptr[:, u * 128 : (u + 1) * 128],
                            vtmp[:, u * 128 : (u + 1) * 128],
                            ident[:],
                        )
                    for u in range(QG // 128):
                        ki = (g2 * 1024 + h * QG) // 128 + u
                        nc.scalar.copy(v_sb[ki][:], ptr[:, u * 128 : (u + 1) * 128])

            # ================= attention (transposed) =================
            if _STAGES in ("attn", "all"):
                for j in range(HPC):
                    for g in range(NG):
                        q0 = g * QG
                        nki = 4 * g + 4 if causal else NKB
                        aT = pool.tile([128, NKB * QG], f32r, tag="aT", name="aT", bufs=2)
                        rs = pp.tile([128, QG], f32, tag="P1", name="rs")
                        for ki in range(nki):
                            sc = pp.tile(
                                [128, QG], f32, tag=f"P{2 + ki % 2}", name="sc"
                            )
                            nc.tensor.matmul(
                                sc[:],
                                kT[:, ki * 128 : (ki + 1) * 128],
                                qT[j][:, q0 : q0 + QG],
                                start=True,
                                stop=True,
                            )
                            dst = aT[:, ki * QG : (ki + 1) * QG]
                            if causal:
                                nc.scalar.activation(dst, sc[:], AF.Exp, scale=SCALE)
                                if ki >= 4 * g:
                                    p = ki - 4 * g
                                    nc.vector.tensor_mul(
                                        dst, dst, dmask[:, p * QG : (p + 1) * QG]
                                    )
                            else:
                                mt = work.tile([128, QG], f32, tag="mt", name="mt")
                                nc.sync.dma_start(
                                    out=mt[:],
                                    in_=mask_d[b, ki * 128 : (ki + 1) * 128, q0 : q0 + QG],
                                )
                                ms = work.tile([128, QG], f32, tag="ms", name="ms")
                                nc.vector.tensor_add(ms[:], sc[:], mt[:])
                                nc.scalar.activation(dst, ms[:], AF.Exp, scale=SCALE)
                        # broadcast row sums to all partitions with an
                        # all-ones matmul, then 1/x = exp(-ln(x)) on ScalarE
                        for ki in range(nki):
                            nc.tensor.matmul(
                                rs[:],
                                ones[:],
                                aT[:, ki * QG : (ki + 1) * QG],
                                start=(ki == 0),
                                stop=(ki == nki - 1),
                            )
                        nl = work.tile([128, QG], f32, tag="nl", name="nl", bufs=1)
                        nc.scalar.activation(nl[:], rs[:], AF.Ln)
                        rb = work.tile([128, QG], f32, tag="rb", name="rb")
                        nc.scalar.activation(rb[:], nl[:], AF.Exp, scale=-1.0)
                        for ki in range(nki):
                            dst = aT[:, ki * QG : (ki + 1) * QG]
                            nc.gpsimd.tensor_tensor(
                                out=dst, in0=dst, in1=rb[:], op=mybir.AluOpType.mult
                            )
                        # one strided DMA writes the whole group's attn slice
                        nc.sync.dma_start(
                            out=attn_d[b, j, 0 : nki * 128, q0 : q0 + QG].rearrange(
                                "(k p) c -> p k c", p=128
                            ),
                            in_=aT[:, : nki * QG].rearrange("p (k c) -> p k c", c=QG),
                        )
                        # o^T accumulation: oT[j][:, q0:q0+QG] += v[ki].T @ aT[ki]
                        po = pp.tile([128, QG], f32, tag="P0", name="po")
                        for ki in range(nki):
                            nc.tensor.matmul(
                                po[:],
                                v_sb[ki][:],
                                aT[:, ki * QG : (ki + 1) * QG],
                                start=(ki == 0),
                                stop=(ki == nki - 1),
                            )
                        nc.vector.tensor_copy(oT[j][:, q0 : q0 + QG], po[:])

            # ================= output projection (partial) =================
            if _STAGES == "all":
                for ti in range(S // 128):
                    ot = work.tile([128, D], f32, tag="ot", name="ot", bufs=1)
                    pw = [
                        pp.tile([128, QG], f32, tag=f"P{k}", name=f"pw{k}")
                        for k in range(4)
                    ]
                    for j in range(HPC):
                        for cg in range(D // QG):
                            nc.tensor.matmul(
                                pw[cg][:],
                                oT[j][:, ti * 128 : (ti + 1) * 128],
                                wo_sb[j][:, cg * QG : (cg + 1) * QG],
                                start=(j == 0),
                                stop=(j == HPC - 1),
                            )
                    for cg in range(D // QG):
                        if cg % 2 == 0:
                            nc.vector.tensor_copy(ot[:, cg * QG : (cg + 1) * QG], pw[cg][:])
                        else:
                            nc.scalar.copy(ot[:, cg * QG : (cg + 1) * QG], pw[cg][:])
                    r0 = b * S + ti * 128
                    nc.sync.dma_start(out=out_d[r0 : r0 + 128, :], in_=ot[:])

    nc.compile()
    return nc


_NC_CACHE = {}


def _get_nc(causal: bool):
    key = (causal, _STAGES)
    if key not in _NC_CACHE:
        _NC_CACHE[key] = _build_nc(causal)
    return _NC_CACHE[key]


def _is_causal_mask(mask: np.ndarray) -> bool:
    if mask.shape != (B, 1, S, S):
        return False
    expect = np.where(
        np.tril(np.ones((S, S), dtype=bool)), np.float32(0.0), np.float32(NEG)
    )
    return all(np.array_equal(mask[b, 0], expect) for b in range(B))


def _make_dmask():
    """dmask[p][r, u*128 + c] for kt sub-position p vs qt sub-block u."""
    dm = np.zeros((4, 128, QG), dtype=np.float32)
    triu = np.triu(np.ones((128, 128), dtype=np.float32))  # keep kt<=qt
    for p in range(4):
        for u in range(4):
            if p < u:
                dm[p][:, u * 128 : (u + 1) * 128] = 1.0
            elif p == u:
                dm[p][:, u * 128 : (u + 1) * 128] = triu
    return dm


def _prep_in_maps(x, cos, sin, attention_mask, Wq, Wk, Wv, Wo, causal):
    xT = np.ascontiguousarray(x.reshape(T, D).T)
    cosT = np.ascontiguousarray(cos.reshape(T, HD).T)
    sinT = np.ascontiguousarray(sin.reshape(T, HD).T)
    ident = np.eye(128, dtype=np.float32)
    ones = np.ones((128, 128), dtype=np.float32)
    dmask = _make_dmask()

    in_maps = []
    for c in range(NCORES):
        kvh = c // 2
        im = {
            "xT": xT,
            "cosT": cosT,
            "sinT": sinT,
            "wq": np.ascontiguousarray(Wq[:, c * HPC * HD : (c + 1) * HPC * HD]),
            "wk": np.ascontiguousarray(Wk[:, kvh * HD : (kvh + 1) * HD]),
            "wv": np.ascontiguousarray(Wv[:, kvh * HD : (kvh + 1) * HD]),
            "wo": np.ascontiguousarray(Wo[c * HPC * HD : (c + 1) * HPC * HD, :]),
            "ident": ident,
            "ones": ones,
            "dmask": dmask,
        }
        if not causal:
            im["maskT"] = np.ascontiguousarray(
                np.swapaxes(attention_mask[:, 0], 1, 2) / np.float32(SCALE)
            )
        in_maps.append(im)
    return in_maps


def kernel(x, cos, sin, attention_mask, Wq, Wk, Wv, Wo):
    from concourse.bass_utils import run_bass_kernel_spmd

    x = np.asarray(x, dtype=np.float32)
    cos = np.asarray(cos, dtype=np.float32)
    sin = np.asarray(sin, dtype=np.float32)
    attention_mask = np.asarray(attention_mask, dtype=np.float32)
    Wq = np.asarray(Wq, dtype=np.float32)
    Wk = np.asarray(Wk, dtype=np.float32)
    Wv = np.asarray(Wv, dtype=np.float32)
    Wo = np.asarray(Wo, dtype=np.float32)

    causal = _is_causal_mask(attention_mask)
    nc = _get_nc(causal)
    in_maps = _prep_in_maps(x, cos, sin, attention_mask, Wq, Wk, Wv, Wo, causal)

    res = run_bass_kernel_spmd(nc, in_maps, list(range(NCORES))).results

    attn = np.concatenate(
        [np.swapaxes(res[c]["attnT"], 2, 3) for c in range(NCORES)], axis=1
    )
    attn = np.ascontiguousarray(attn)
    out = res[0]["out_partial"].astype(np.float32)
    for c in range(1, NCORES):
        out = out + res[c]["out_partial"]
    out = out.reshape(B, S, D)
    return out, attn
